# revision 23
# baseline (speedup 1.0000x reference)
"""Trainium2 Bass kernel for AtomEmbedding:
    h = LayerNorm(emb[z] + W2 @ silu(W1 @ x + b1) + b2) * gamma + beta

v3 design (PE-bound, sustained 2.4GHz p-state):
  Trace analysis of v2 (626us) showed the PE ~100% occupied (LDWEIGHTS
  231us + MATMUL 401us) but stuck at the 1.2GHz mid p-state: TRN2's PE
  only ramps to 2.4GHz after ~3us of gap-free execution, and v2's PE
  stalled briefly every group (PSUM recycling waited on the full
  stats->epilogue->normalize chain reading PSUM).

  v3 makes the PE the strict bottleneck (5528 cyc/group = 2.30us @2.4GHz)
  and keeps every other engine under that:
  - PE: mm1 (pT = [w1;b1]^T [x;1]^T), mm2 (2 k-chunks), one-hot gather
    matmul (emb+b2 streamed against one-hot^T stationary), per 128-atom
    tile into PSUM hp [128,4,256] (2 banks, bufs=2; pT 2 banks, bufs=2).
  - PSUM is freed IMMEDIATELY by a single ACT copy pass hp -> SBUF bf16,
    so the PE never waits on PSUM recycling. ACT total: silu + copy.
  - LayerNorm from the bf16 copy: per-tile bn_stats/bn_aggr on DVE
    (cheap SBUF access), rsqrt(var+eps) via int-seed (DVE) + Newton on
    the otherwise-idle GpSimd (plain tensor_tensor only -- walrus rejects
    TensorScalarPtr on Pool), cc = -mu*rs on GpSimd, and norm
    o = h*rs + cc as one tensor_scalar per tile on DVE whose bf16-SBUF
    operands enable the 4x DVE perf mode (per-partition scalars exempt).
  - Loads (xT, ohT) and stores batched over 2 groups per DMA to halve the
    Sync-sequencer DGE config cost (~565ns/DMA).
  - 7-stage software pipeline: load | copy(-3) | stats(-4) | coef(-5) |
    norm+store(-6) | mm1+silu(-1) | mm2+gather(-2), emitted so every
    cross-engine dependency has ~a full group period of slack (ACT order:
    copy then silu).
  - Output bf16 (host casts back to f32); b2 folded into emb host-side.
"""

import os
import sys

import numpy as np

for _p in ("/opt/trn_rl_repo", "/opt/pypackages"):
    if _p not in sys.path and os.path.isdir(_p):
        sys.path.append(_p)

N = 524288
D = 256
NT = 100  # number of atom types
NCORES = 8
NPC = N // NCORES  # atoms per core
A = int(os.environ.get("ATOMEMB_A", "512"))  # atoms per group
TPG = A // 128  # 128-atom tiles per group
EPS = 1e-5

# knobs (defaults = the design; env lets HW A/B without editing)
OUT_BF16 = os.environ.get("ATOMEMB_OUT_BF16", "1") == "1"
NCOPY_ACT = int(os.environ.get("ATOMEMB_NCOPY_ACT", "4"))  # copy tiles on ACT
NORM_ACT = int(os.environ.get("ATOMEMB_NORM_ACT", "0"))  # norm tiles on ACT
NEWTON_ITERS = int(os.environ.get("ATOMEMB_NEWTON", "1"))
# center emb rows + w2 rows over d host-side: mean(h) == 0 by construction
# (var is translation-invariant), so cc = -mu*rs vanishes and the norm is a
# single-scalar multiply
CENTER = os.environ.get("ATOMEMB_CENTER", "1") == "1"
BUFS_IN = int(os.environ.get("ATOMEMB_BIN", "3"))  # 2-group load tiles
BUFS_S = int(os.environ.get("ATOMEMB_BS", "3"))
BUFS_H = int(os.environ.get("ATOMEMB_BH", "4"))
BUFS_O = int(os.environ.get("ATOMEMB_BO", "3"))  # 2-group store tiles
PSA_BUFS = int(os.environ.get("ATOMEMB_PSA", "2"))
PSB_BUFS = int(os.environ.get("ATOMEMB_PSB", "2"))

_MODULE_CACHE: dict = {}


def _build_module(npc: int, apply_affine: bool, sim_safe_silu: bool = False):
    """Build + compile the Bass module for one core's slice (npc atoms).

    sim_safe_silu: CoreSim doesn't implement the Silu activation; when True,
    emit Sigmoid + multiply instead (slower, only used for simulation runs).
    """
    from contextlib import ExitStack

    import concourse.bacc as bacc
    import concourse.tile as tile
    from concourse import mybir

    f32 = mybir.dt.float32
    bf16 = mybir.dt.bfloat16
    i32 = mybir.dt.int32
    out_dt = bf16 if OUT_BF16 else f32
    TT = mybir.AluOpType

    ngroups = npc // A
    assert npc % A == 0 and ngroups % 2 == 0

    nc = bacc.Bacc(
        "TRN2",
        target_bir_lowering=False,
        debug=False,
        enable_asserts=False,
        num_devices=NCORES,
    )

    # Per-core inputs (host pre-transposed / folded):
    #   xT:   [128, npc] rows = (x0, x1, x2, 1, 0...) -> moving operand of mm1
    #   ohT:  [NT, npc] one-hot^T                   -> stationary of gather-mm
    #   w1c:  [128, D]  [w1; b1; 0...]              -> stationary of mm1
    #   w2a:  [128, 2, D] w2 split into two k-chunks -> moving of mm2
    #   emba: [NT, D]   emb + b2                    -> moving of gather-mm
    # mm1 is zero-padded to K=128 (both operands): same column count, but
    # the PE stays at full MAC utilization -- the K=4 matmul empirically
    # pins the PE's DVFS at the 1.2GHz mid p-state, while the trace's
    # mm1-free drain phase ran at 2.4GHz.
    xT = nc.dram_tensor("xT", [128, npc], bf16, kind="ExternalInput")
    ohT = nc.dram_tensor("ohT", [NT, npc], bf16, kind="ExternalInput")
    w1c = nc.dram_tensor("w1c", [128, D], bf16, kind="ExternalInput")
    w2a = nc.dram_tensor("w2a", [128, 2, D], bf16, kind="ExternalInput")
    emba = nc.dram_tensor("emba", [NT, D], bf16, kind="ExternalInput")
    if apply_affine:
        gmb = nc.dram_tensor("gmb", [128, D], f32, kind="ExternalInput")
        btb = nc.dram_tensor("btb", [128, D], f32, kind="ExternalInput")
    out = nc.dram_tensor("out", [npc, D], out_dt, kind="ExternalOutput")

    with tile.TileContext(nc) as tc:
        with ExitStack() as ctx:
            consts = ctx.enter_context(tc.tile_pool(name="consts", bufs=1))
            xpool = ctx.enter_context(tc.tile_pool(name="xpool", bufs=BUFS_IN))
            ohpool = ctx.enter_context(tc.tile_pool(name="ohpool", bufs=BUFS_IN))
            spool = ctx.enter_context(tc.tile_pool(name="spool", bufs=BUFS_S))
            hpool = ctx.enter_context(tc.tile_pool(name="hpool", bufs=BUFS_H))
            stpool = ctx.enter_context(tc.tile_pool(name="stpool", bufs=4))
            opool = ctx.enter_context(tc.tile_pool(name="opool", bufs=BUFS_O))
            psA = ctx.enter_context(
                tc.tile_pool(name="psA", bufs=PSA_BUFS, space="PSUM"))
            psB = ctx.enter_context(
                tc.tile_pool(name="psB", bufs=PSB_BUFS, space="PSUM"))

            # ---- load constants once ----
            sb_w1 = consts.tile([128, D], bf16)
            nc.sync.dma_start(out=sb_w1[:], in_=w1c[:])

            sb_w2 = consts.tile([128, 2, D], bf16)
            nc.sync.dma_start(out=sb_w2[:], in_=w2a[:])
            sb_emb = consts.tile([NT, D], bf16)
            nc.sync.dma_start(out=sb_emb[:], in_=emba[:])
            # epilogue const tiles (GpSimd only does plain tensor_tensor)
            sb_magic_i = consts.tile([128, TPG], i32)
            nc.vector.memset(sb_magic_i[:], 0x5F3759DF)
            sb_one_i = consts.tile([128, TPG], i32)
            nc.vector.memset(sb_one_i[:], 1)
            sb_eps = consts.tile([128, TPG], f32)
            nc.vector.memset(sb_eps[:], EPS)
            sb_nhalf = consts.tile([128, TPG], f32)
            nc.vector.memset(sb_nhalf[:], -0.5)
            sb_1p5 = consts.tile([128, TPG], f32)
            nc.vector.memset(sb_1p5[:], 1.5)
            sb_neg1 = consts.tile([128, TPG], f32)
            nc.vector.memset(sb_neg1[:], -1.0)
            if apply_affine:
                sb_gmb = consts.tile([128, D], f32)
                nc.sync.dma_start(out=sb_gmb[:], in_=gmb[:])
                sb_btb = consts.tile([128, D], f32)
                nc.sync.dma_start(out=sb_btb[:], in_=btb[:])

            # Software-pipelined emission (engines execute their streams in
            # order, so emission order IS the schedule). Stage skews give
            # every cross-engine dependency ~a full group period of slack:
            #   it: load(it,it+1) | post1(it-3) | post2(it-4) | front(it-1)
            #       | back(it-2)
            # ACT order per iteration: copy(it-3), sqrt(it-4), silu(it-1)
            # so silu never queues behind a not-yet-ready op.
            live: dict = {}

            def stage_load(g):
                # one DMA covers groups g and g+1 (halves SP DGE config cost)
                if g % 2 == 1:
                    return
                a0 = g * A
                xt = xpool.tile([128, 2 * A], bf16, tag="xt")
                nc.sync.dma_start(out=xt[:], in_=xT[:, a0 : a0 + 2 * A])
                oh = ohpool.tile([NT, 2 * A], bf16, tag="oh")
                nc.sync.dma_start(out=oh[:], in_=ohT[:, a0 : a0 + 2 * A])
                live[g] = {"xt": xt, "oh": oh}
                live[g + 1] = {"xt": xt, "oh": oh, "off": A}

            def stage_front(g):
                o = live[g].get("off", 0)
                xt = live[g]["xt"]
                # mm1: p^T [128, 2, A] (D on partitions, one PSUM tile)
                pT = psA.tile([128, 2, A], f32, tag="pT")
                nc.tensor.matmul(pT[:, 0, :], sb_w1[:, 0:128],
                                 xt[:, o : o + A], start=True, stop=True)
                nc.tensor.matmul(pT[:, 1, :], sb_w1[:, 128:256],
                                 xt[:, o : o + A], start=True, stop=True)
                # silu on ACT (single op over FD=2A)
                s = spool.tile([128, 2, A], bf16, tag="s")
                if sim_safe_silu == "timing":
                    # Sigmoid only: wrong values, same ACT cost as Silu --
                    # used by simtime.py for schedule-timing prediction.
                    nc.scalar.activation(s[:], pT[:],
                                         mybir.ActivationFunctionType.Sigmoid)
                elif sim_safe_silu:
                    sg = spool.tile([128, 2, A], f32, tag="sg")
                    nc.scalar.activation(sg[:], pT[:],
                                         mybir.ActivationFunctionType.Sigmoid)
                    nc.vector.tensor_mul(s[:], sg[:], pT[:])
                else:
                    nc.scalar.activation(s[:], pT[:],
                                         mybir.ActivationFunctionType.Silu)
                live[g]["s"] = s

            def stage_back(g):
                o = live[g].get("off", 0)
                s, oh = live[g]["s"], live[g]["oh"]
                # mm2 + embedding gather into one PSUM tile [128, 4, 256]
                # (2 banks; tile slices at 1KB offsets never cross a bank)
                hp = psB.tile([128, TPG, D], f32, tag="hp")
                for t in range(TPG):
                    c = t * 128
                    h = hp[:, t, :]
                    nc.tensor.matmul(h, s[:, 0, c : c + 128],
                                     sb_w2[:, 0, :], start=True, stop=False)
                    nc.tensor.matmul(h, s[:, 1, c : c + 128],
                                     sb_w2[:, 1, :], start=False, stop=False)
                    nc.tensor.matmul(h, oh[:, o + c : o + c + 128],
                                     sb_emb[:], start=False, stop=True)
                live[g]["hp"] = hp

            def stage_copy(g):
                hp = live[g]["hp"]
                # PSUM -> SBUF bf16 copy (ACT, one op over [128, TPG*D]);
                # releases hp so the PE never waits on PSUM recycling.
                hsb = hpool.tile([128, TPG, D], bf16, tag="hsb")
                ca = max(0, min(TPG, NCOPY_ACT))
                if ca:
                    nc.scalar.activation(hsb[:, 0:ca, :], hp[:, 0:ca, :],
                                         mybir.ActivationFunctionType.Copy)
                if ca < TPG:
                    nc.vector.tensor_scalar(
                        out=hsb[:, ca:TPG, :], in0=hp[:, ca:TPG, :],
                        scalar1=1.0, scalar2=None, op0=TT.mult)
                live[g]["hsb"] = hsb

            def stage_stat(g):
                hsb = live[g]["hsb"]
                # LayerNorm stats from the bf16 copy (cheaper SBUF access;
                # bn_stats is single-group: one op per 128-atom tile; bn_aggr
                # output must be exactly 2 wide).
                stg = stpool.tile([128, TPG, 6], f32, tag="stg")
                for t in range(TPG):
                    nc.vector.bn_stats(out=stg[:, t, :], in_=hsb[:, t, :])
                mv = stpool.tile([128, TPG, 2], f32, tag="mv")
                for t in range(TPG):
                    nc.vector.bn_aggr(out=mv[:, t, :], in_=stg[:, t, :])
                # w = var + eps on GpSimd (idle engine; plain tensor_tensor)
                w = stpool.tile([128, TPG], f32, tag="w")
                nc.gpsimd.tensor_tensor(out=w[:], in0=mv[:, :, 1],
                                        in1=sb_eps[:], op=TT.add)
                live[g]["mv"] = mv
                live[g]["w"] = w

            def stage_coef(g):
                mv, w = live[g]["mv"], live[g]["w"]
                # rs = rsqrt(var+eps): 0x5f3759df seed (int shift is
                # DVE-only) + Newton on GpSimd; cc = -mu*rs on GpSimd.
                y = stpool.tile([128, TPG], f32, tag="y")
                t1 = stpool.tile([128, TPG], f32, tag="t1")
                nc.vector.tensor_tensor(
                    out=t1[:].bitcast(i32), in0=w[:].bitcast(i32),
                    in1=sb_one_i[:], op=TT.logical_shift_right)
                nc.vector.tensor_tensor(
                    out=y[:].bitcast(i32), in0=sb_magic_i[:],
                    in1=t1[:].bitcast(i32), op=TT.subtract)
                for _ in range(NEWTON_ITERS):
                    nc.gpsimd.tensor_tensor(out=t1[:], in0=y[:], in1=y[:],
                                            op=TT.mult)
                    nc.gpsimd.tensor_tensor(out=t1[:], in0=t1[:], in1=w[:],
                                            op=TT.mult)
                    nc.gpsimd.tensor_tensor(out=t1[:], in0=t1[:],
                                            in1=sb_nhalf[:], op=TT.mult)
                    nc.gpsimd.tensor_tensor(out=t1[:], in0=t1[:],
                                            in1=sb_1p5[:], op=TT.add)
                    nc.gpsimd.tensor_tensor(out=y[:], in0=t1[:], in1=y[:],
                                            op=TT.mult)
                cc = stpool.tile([128, TPG], f32, tag="cc")
                nc.gpsimd.tensor_tensor(out=cc[:], in0=mv[:, :, 0],
                                        in1=y[:], op=TT.mult)
                nc.gpsimd.tensor_tensor(out=cc[:], in0=cc[:],
                                        in1=sb_neg1[:], op=TT.mult)
                live[g]["rs"] = y
                live[g]["cc"] = cc

            def stage_norm(g):
                a0 = g * A
                hsb = live[g]["hsb"]
                rs, cc = live[g]["rs"], live[g]["cc"]
                # normalize o = h*rs + cc from the bf16 SBUF copy; bf16-SBUF
                # operands enable the DVE 4x perf mode (scalars exempt).
                if g % 2 == 0:
                    og = opool.tile([128, 2, TPG, D], out_dt, tag="og")
                    live[g]["og"] = og
                else:
                    og = live[g - 1]["og"]
                gs = g % 2
                for t in range(TPG):
                    if t < TPG - NORM_ACT:
                        nc.vector.tensor_scalar(
                            out=og[:, gs, t, :], in0=hsb[:, t, :],
                            scalar1=rs[:, t : t + 1], scalar2=cc[:, t : t + 1],
                            op0=TT.mult, op1=TT.add)
                    else:
                        nc.scalar.activation(
                            og[:, gs, t, :], hsb[:, t, :],
                            mybir.ActivationFunctionType.Identity,
                            bias=cc[:, t : t + 1], scale=rs[:, t : t + 1])
                    if apply_affine:
                        nc.vector.tensor_mul(og[:, gs, t, :],
                                             og[:, gs, t, :], sb_gmb[:])
                        nc.vector.tensor_add(og[:, gs, t, :],
                                             og[:, gs, t, :], sb_btb[:])
                if g % 2 == 1:
                    # one store covers groups g-1 and g:
                    # DRAM row (g-1)*A + u*A + t*128 + p  <-  og[p, u, t, :]
                    out_view = out[a0 - A : a0 + A, :].rearrange(
                        "(u t p) d -> p u t d", p=128, u=2)
                    nc.sync.dma_start(out=out_view, in_=og[:])
                    del live[g - 1]
                    del live[g]

            for it in range(ngroups + 6):
                if it < ngroups:
                    stage_load(it)
                if 0 <= it - 3 < ngroups:
                    stage_copy(it - 3)
                if 0 <= it - 4 < ngroups:
                    stage_stat(it - 4)
                if 0 <= it - 5 < ngroups:
                    stage_coef(it - 5)
                if 0 <= it - 6 < ngroups:
                    stage_norm(it - 6)
                if 0 <= it - 1 < ngroups:
                    stage_front(it - 1)
                if 0 <= it - 2 < ngroups:
                    stage_back(it - 2)

    nc.compile()
    return nc


def _get_module(npc: int, apply_affine: bool, sim_safe_silu: bool = False):
    key = (npc, apply_affine, sim_safe_silu)
    if key not in _MODULE_CACHE:
        _MODULE_CACHE[key] = _build_module(npc, apply_affine, sim_safe_silu)
    return _MODULE_CACHE[key]


def _prep_inputs(z, x, emb, w1, b1, w2, b2, gamma, beta, npc, apply_affine):
    """Host-side folding/transposes; returns per-core in_maps."""
    import ml_dtypes

    st = ml_dtypes.bfloat16

    z = np.asarray(z)
    x = np.asarray(x, dtype=np.float32)
    n = z.shape[0]

    xT = np.zeros((128, n), dtype=np.float32)
    xT[0:3] = x.T
    xT[3] = 1.0
    xT = xT.astype(st)
    zi = np.asarray(z).astype(np.int64)
    ohT = (zi[None, :] == np.arange(NT, dtype=np.int64)[:, None]).astype(st)
    w1a = np.zeros((128, D), dtype=np.float32)
    w1a[0:3] = np.asarray(w1, np.float32)
    w1a[3] = np.asarray(b1, np.float32).reshape(D)
    w1c = w1a.astype(st)
    w2f = np.asarray(w2, np.float32)
    w2a = np.stack([w2f[0:128], w2f[128:256]], axis=1).astype(st)
    emba = (np.asarray(emb, np.float32)
            + np.asarray(b2, np.float32).reshape(1, D)).astype(st)

    common = {"w1c": w1c, "w2a": w2a, "emba": emba}
    if apply_affine:
        common["gmb"] = np.broadcast_to(
            np.asarray(gamma, np.float32).reshape(1, D), (128, D)).copy()
        common["btb"] = np.broadcast_to(
            np.asarray(beta, np.float32).reshape(1, D), (128, D)).copy()

    in_maps = []
    for c in range(NCORES):
        s = slice(c * npc, (c + 1) * npc)
        m = {"xT": np.ascontiguousarray(xT[:, s]),
             "ohT": np.ascontiguousarray(ohT[:, s]), **common}
        in_maps.append(m)
    return in_maps


def _run(in_maps, nc, trace=False):
    from concourse.bass_interp import get_hw_module
    from concourse.bass_utils import run_bass_kernel_spmd

    old_m = nc.m
    nc.m = get_hw_module(nc.m)
    try:
        res = run_bass_kernel_spmd(
            nc, in_maps, core_ids=list(range(NCORES)), trace=trace
        )
    finally:
        nc.m = old_m
    return res


def kernel(z, x, emb, w1, b1, w2, b2, gamma, beta):
    z = np.asarray(z)
    x = np.asarray(x)
    assert z.shape[0] == N and x.shape == (N, 3), (z.shape, x.shape)

    apply_affine = not (
        np.all(np.asarray(gamma) == 1.0) and np.all(np.asarray(beta) == 0.0)
    )
    nc = _get_module(NPC, apply_affine)
    in_maps = _prep_inputs(z, x, emb, w1, b1, w2, b2, gamma, beta,
                           NPC, apply_affine)
    res = _run(in_maps, nc, trace=False)
    out = np.concatenate([np.asarray(r["out"]) for r in res.results], axis=0)
    return out.astype(np.float32)


# revision 27
# speedup vs baseline: 1.1679x; 1.1679x over previous
"""Trainium2 Bass kernel for AtomEmbedding:
    h = LayerNorm(emb[z] + W2 @ silu(W1 @ x + b1) + b2) * gamma + beta

v3 design (PE-bound, sustained 2.4GHz p-state):
  Trace analysis of v2 (626us) showed the PE ~100% occupied (LDWEIGHTS
  231us + MATMUL 401us) but stuck at the 1.2GHz mid p-state: TRN2's PE
  only ramps to 2.4GHz after ~3us of gap-free execution, and v2's PE
  stalled briefly every group (PSUM recycling waited on the full
  stats->epilogue->normalize chain reading PSUM).

  v3 makes the PE the strict bottleneck (5528 cyc/group = 2.30us @2.4GHz)
  and keeps every other engine under that:
  - PE: mm1 (pT = [w1;b1]^T [x;1]^T), mm2 (2 k-chunks), one-hot gather
    matmul (emb+b2 streamed against one-hot^T stationary), per 128-atom
    tile into PSUM hp [128,4,256] (2 banks, bufs=2; pT 2 banks, bufs=2).
  - PSUM is freed IMMEDIATELY by a single ACT copy pass hp -> SBUF bf16,
    so the PE never waits on PSUM recycling. ACT total: silu + copy.
  - LayerNorm from the bf16 copy: per-tile bn_stats/bn_aggr on DVE
    (cheap SBUF access), rsqrt(var+eps) via int-seed (DVE) + Newton on
    the otherwise-idle GpSimd (plain tensor_tensor only -- walrus rejects
    TensorScalarPtr on Pool), cc = -mu*rs on GpSimd, and norm
    o = h*rs + cc as one tensor_scalar per tile on DVE whose bf16-SBUF
    operands enable the 4x DVE perf mode (per-partition scalars exempt).
  - Loads (xT, ohT) and stores batched over 2 groups per DMA to halve the
    Sync-sequencer DGE config cost (~565ns/DMA).
  - 7-stage software pipeline: load | copy(-3) | stats(-4) | coef(-5) |
    norm+store(-6) | mm1+silu(-1) | mm2+gather(-2), emitted so every
    cross-engine dependency has ~a full group period of slack (ACT order:
    copy then silu).
  - Output bf16 (host casts back to f32); b2 folded into emb host-side.
"""

import os
import sys

import numpy as np

for _p in ("/opt/trn_rl_repo", "/opt/pypackages"):
    if _p not in sys.path and os.path.isdir(_p):
        sys.path.append(_p)

N = 524288
D = 256
NT = 100  # number of atom types
NCORES = 8
NPC = N // NCORES  # atoms per core
A = int(os.environ.get("ATOMEMB_A", "512"))  # atoms per group
TPG = A // 128  # 128-atom tiles per group
EPS = 1e-5

# knobs (defaults = the design; env lets HW A/B without editing)
OUT_BF16 = os.environ.get("ATOMEMB_OUT_BF16", "1") == "1"
NCOPY_ACT = int(os.environ.get("ATOMEMB_NCOPY_ACT", "4"))  # copy tiles on ACT
NORM_ACT = int(os.environ.get("ATOMEMB_NORM_ACT", "0"))  # norm tiles on ACT
NEWTON_ITERS = int(os.environ.get("ATOMEMB_NEWTON", "1"))
# center emb rows + w2 rows over d host-side: mean(h) == 0 by construction
# (var is translation-invariant), so cc = -mu*rs vanishes and the norm is a
# single-scalar multiply
CENTER = os.environ.get("ATOMEMB_CENTER", "1") == "1"
BUFS_IN = int(os.environ.get("ATOMEMB_BIN", "3"))  # 2-group load tiles
BUFS_S = int(os.environ.get("ATOMEMB_BS", "3"))
BUFS_H = int(os.environ.get("ATOMEMB_BH", "4"))
BUFS_O = int(os.environ.get("ATOMEMB_BO", "3"))  # 2-group store tiles
PSA_BUFS = int(os.environ.get("ATOMEMB_PSA", "2"))
PSB_BUFS = int(os.environ.get("ATOMEMB_PSB", "2"))

_MODULE_CACHE: dict = {}


def _build_module(npc: int, apply_affine: bool, sim_safe_silu: bool = False):
    """Build + compile the Bass module for one core's slice (npc atoms).

    sim_safe_silu: CoreSim doesn't implement the Silu activation; when True,
    emit Sigmoid + multiply instead (slower, only used for simulation runs).
    """
    from contextlib import ExitStack

    import concourse.bacc as bacc
    import concourse.tile as tile
    from concourse import mybir

    f32 = mybir.dt.float32
    bf16 = mybir.dt.bfloat16
    i32 = mybir.dt.int32
    out_dt = bf16 if OUT_BF16 else f32
    TT = mybir.AluOpType

    ngroups = npc // A
    assert npc % A == 0 and ngroups % 2 == 0

    nc = bacc.Bacc(
        "TRN2",
        target_bir_lowering=False,
        debug=False,
        enable_asserts=False,
        num_devices=NCORES,
    )

    # Per-core inputs (host pre-transposed / folded):
    #   xT:   [128, npc] rows = (x0, x1, x2, 1, 0...) -> moving operand of mm1
    #   ohT:  [NT, npc] one-hot^T                   -> stationary of gather-mm
    #   w1c:  [128, D]  [w1; b1; 0...]              -> stationary of mm1
    #   w2a:  [128, 2, D] w2 split into two k-chunks -> moving of mm2
    #   emba: [NT, D]   emb + b2                    -> moving of gather-mm
    # mm1 is zero-padded to K=128 (both operands): same column count, but
    # the PE stays at full MAC utilization -- the K=4 matmul empirically
    # pins the PE's DVFS at the 1.2GHz mid p-state, while the trace's
    # mm1-free drain phase ran at 2.4GHz.
    xT = nc.dram_tensor("xT", [128, npc], bf16, kind="ExternalInput")
    ohT = nc.dram_tensor("ohT", [NT, npc], bf16, kind="ExternalInput")
    w1c = nc.dram_tensor("w1c", [128, D], bf16, kind="ExternalInput")
    w2a = nc.dram_tensor("w2a", [128, 2, D], bf16, kind="ExternalInput")
    emba = nc.dram_tensor("emba", [NT, D], bf16, kind="ExternalInput")
    if apply_affine:
        gmb = nc.dram_tensor("gmb", [128, D], f32, kind="ExternalInput")
        btb = nc.dram_tensor("btb", [128, D], f32, kind="ExternalInput")
    out = nc.dram_tensor("out", [npc, D], out_dt, kind="ExternalOutput")

    with tile.TileContext(nc) as tc:
        with ExitStack() as ctx:
            consts = ctx.enter_context(tc.tile_pool(name="consts", bufs=1))
            xpool = ctx.enter_context(tc.tile_pool(name="xpool", bufs=BUFS_IN))
            ohpool = ctx.enter_context(tc.tile_pool(name="ohpool", bufs=BUFS_IN))
            spool = ctx.enter_context(tc.tile_pool(name="spool", bufs=BUFS_S))
            hpool = ctx.enter_context(tc.tile_pool(name="hpool", bufs=BUFS_H))
            stpool = ctx.enter_context(tc.tile_pool(name="stpool", bufs=4))
            opool = ctx.enter_context(tc.tile_pool(name="opool", bufs=BUFS_O))
            psA = ctx.enter_context(
                tc.tile_pool(name="psA", bufs=PSA_BUFS, space="PSUM"))
            psB = ctx.enter_context(
                tc.tile_pool(name="psB", bufs=PSB_BUFS, space="PSUM"))

            # ---- load constants once ----
            sb_w1 = consts.tile([128, D], bf16)
            nc.sync.dma_start(out=sb_w1[:], in_=w1c[:])

            sb_w2 = consts.tile([128, 2, D], bf16)
            nc.sync.dma_start(out=sb_w2[:], in_=w2a[:])
            sb_emb = consts.tile([NT, D], bf16)
            nc.sync.dma_start(out=sb_emb[:], in_=emba[:])
            # epilogue const tiles (GpSimd only does plain tensor_tensor)
            sb_magic_i = consts.tile([128, TPG], i32)
            nc.vector.memset(sb_magic_i[:], 0x5F3759DF)
            sb_one_i = consts.tile([128, TPG], i32)
            nc.vector.memset(sb_one_i[:], 1)
            sb_eps = consts.tile([128, TPG], f32)
            nc.vector.memset(sb_eps[:], EPS)
            sb_nhalf = consts.tile([128, TPG], f32)
            nc.vector.memset(sb_nhalf[:], -0.5)
            sb_1p5 = consts.tile([128, TPG], f32)
            nc.vector.memset(sb_1p5[:], 1.5)
            sb_neg1 = consts.tile([128, TPG], f32)
            nc.vector.memset(sb_neg1[:], -1.0)
            if apply_affine:
                sb_gmb = consts.tile([128, D], f32)
                nc.sync.dma_start(out=sb_gmb[:], in_=gmb[:])
                sb_btb = consts.tile([128, D], f32)
                nc.sync.dma_start(out=sb_btb[:], in_=btb[:])

            # Software-pipelined emission (engines execute their streams in
            # order, so emission order IS the schedule). Stage skews give
            # every cross-engine dependency ~a full group period of slack:
            #   it: load(it,it+1) | post1(it-3) | post2(it-4) | front(it-1)
            #       | back(it-2)
            # ACT order per iteration: copy(it-3), sqrt(it-4), silu(it-1)
            # so silu never queues behind a not-yet-ready op.
            live: dict = {}

            def stage_load(g):
                # one DMA covers groups g and g+1 (halves SP DGE config cost)
                if g % 2 == 1:
                    return
                a0 = g * A
                xt = xpool.tile([128, 2 * A], bf16, tag="xt")
                nc.sync.dma_start(out=xt[:], in_=xT[:, a0 : a0 + 2 * A])
                oh = ohpool.tile([NT, 2 * A], bf16, tag="oh")
                nc.sync.dma_start(out=oh[:], in_=ohT[:, a0 : a0 + 2 * A])
                live[g] = {"xt": xt, "oh": oh}
                live[g + 1] = {"xt": xt, "oh": oh, "off": A}

            def stage_front(g):
                o = live[g].get("off", 0)
                xt = live[g]["xt"]
                # mm1: p^T [128, 2, A] (D on partitions, one PSUM tile)
                pT = psA.tile([128, 2, A], f32, tag="pT")
                nc.tensor.matmul(pT[:, 0, :], sb_w1[:, 0:128],
                                 xt[:, o : o + A], start=True, stop=True)
                nc.tensor.matmul(pT[:, 1, :], sb_w1[:, 128:256],
                                 xt[:, o : o + A], start=True, stop=True)
                # silu on ACT (single op over FD=2A)
                s = spool.tile([128, 2, A], bf16, tag="s")
                if sim_safe_silu == "timing":
                    # Sigmoid only: wrong values, same ACT cost as Silu --
                    # used by simtime.py for schedule-timing prediction.
                    nc.scalar.activation(s[:], pT[:],
                                         mybir.ActivationFunctionType.Sigmoid)
                elif sim_safe_silu:
                    sg = spool.tile([128, 2, A], f32, tag="sg")
                    nc.scalar.activation(sg[:], pT[:],
                                         mybir.ActivationFunctionType.Sigmoid)
                    nc.vector.tensor_mul(s[:], sg[:], pT[:])
                else:
                    nc.scalar.activation(s[:], pT[:],
                                         mybir.ActivationFunctionType.Silu)
                live[g]["s"] = s

            def stage_back(g):
                o = live[g].get("off", 0)
                s, oh = live[g]["s"], live[g]["oh"]
                # mm2 + embedding gather into one PSUM tile [128, 4, 256]
                # (2 banks; tile slices at 1KB offsets never cross a bank)
                hp = psB.tile([128, TPG, D], f32, tag="hp")
                for t in range(TPG):
                    c = t * 128
                    h = hp[:, t, :]
                    nc.tensor.matmul(h, s[:, 0, c : c + 128],
                                     sb_w2[:, 0, :], start=True, stop=False)
                    nc.tensor.matmul(h, s[:, 1, c : c + 128],
                                     sb_w2[:, 1, :], start=False, stop=False)
                    nc.tensor.matmul(h, oh[:, o + c : o + c + 128],
                                     sb_emb[:], start=False, stop=True)
                live[g]["hp"] = hp

            def stage_copy(g):
                hp = live[g]["hp"]
                # PSUM -> SBUF bf16 copy (ACT, one op over [128, TPG*D]);
                # releases hp so the PE never waits on PSUM recycling.
                hsb = hpool.tile([128, TPG, D], bf16, tag="hsb")
                ca = max(0, min(TPG, NCOPY_ACT))
                if ca:
                    nc.scalar.activation(hsb[:, 0:ca, :], hp[:, 0:ca, :],
                                         mybir.ActivationFunctionType.Copy)
                if ca < TPG:
                    nc.vector.tensor_scalar(
                        out=hsb[:, ca:TPG, :], in0=hp[:, ca:TPG, :],
                        scalar1=1.0, scalar2=None, op0=TT.mult)
                live[g]["hsb"] = hsb

            def stage_stat(g):
                hsb = live[g]["hsb"]
                # LayerNorm stats from the bf16 copy (cheaper SBUF access;
                # bn_stats is single-group: one op per 128-atom tile; bn_aggr
                # output must be exactly 2 wide).
                stg = stpool.tile([128, TPG, 6], f32, tag="stg")
                for t in range(TPG):
                    nc.vector.bn_stats(out=stg[:, t, :], in_=hsb[:, t, :])
                mv = stpool.tile([128, TPG, 2], f32, tag="mv")
                for t in range(TPG):
                    nc.vector.bn_aggr(out=mv[:, t, :], in_=stg[:, t, :])
                # w = var + eps on GpSimd (idle engine; plain tensor_tensor)
                w = stpool.tile([128, TPG], f32, tag="w")
                nc.gpsimd.tensor_tensor(out=w[:], in0=mv[:, :, 1],
                                        in1=sb_eps[:], op=TT.add)
                live[g]["mv"] = mv
                live[g]["w"] = w

            def stage_coef(g):
                mv, w = live[g]["mv"], live[g]["w"]
                # rs = rsqrt(var+eps): 0x5f3759df seed (int shift is
                # DVE-only) + Newton on GpSimd; cc = -mu*rs on GpSimd.
                y = stpool.tile([128, TPG], f32, tag="y")
                t1 = stpool.tile([128, TPG], f32, tag="t1")
                nc.vector.tensor_tensor(
                    out=t1[:].bitcast(i32), in0=w[:].bitcast(i32),
                    in1=sb_one_i[:], op=TT.logical_shift_right)
                nc.vector.tensor_tensor(
                    out=y[:].bitcast(i32), in0=sb_magic_i[:],
                    in1=t1[:].bitcast(i32), op=TT.subtract)
                for _ in range(NEWTON_ITERS):
                    nc.gpsimd.tensor_tensor(out=t1[:], in0=y[:], in1=y[:],
                                            op=TT.mult)
                    nc.gpsimd.tensor_tensor(out=t1[:], in0=t1[:], in1=w[:],
                                            op=TT.mult)
                    nc.gpsimd.tensor_tensor(out=t1[:], in0=t1[:],
                                            in1=sb_nhalf[:], op=TT.mult)
                    nc.gpsimd.tensor_tensor(out=t1[:], in0=t1[:],
                                            in1=sb_1p5[:], op=TT.add)
                    nc.gpsimd.tensor_tensor(out=y[:], in0=t1[:], in1=y[:],
                                            op=TT.mult)
                if CENTER:
                    cc = None
                else:
                    cc = stpool.tile([128, TPG], f32, tag="cc")
                    nc.gpsimd.tensor_tensor(out=cc[:], in0=mv[:, :, 0],
                                            in1=y[:], op=TT.mult)
                    nc.gpsimd.tensor_tensor(out=cc[:], in0=cc[:],
                                            in1=sb_neg1[:], op=TT.mult)
                live[g]["rs"] = y
                live[g]["cc"] = cc

            def stage_norm(g):
                a0 = g * A
                hsb = live[g]["hsb"]
                rs, cc = live[g]["rs"], live[g]["cc"]
                # normalize o = h*rs + cc from the bf16 SBUF copy; bf16-SBUF
                # operands enable the DVE 4x perf mode (scalars exempt).
                if g % 2 == 0:
                    og = opool.tile([128, 2, TPG, D], out_dt, tag="og")
                    live[g]["og"] = og
                else:
                    og = live[g - 1]["og"]
                gs = g % 2
                for t in range(TPG):
                    if t < TPG - NORM_ACT:
                        if CENTER:
                            nc.vector.tensor_scalar(
                                out=og[:, gs, t, :], in0=hsb[:, t, :],
                                scalar1=rs[:, t : t + 1], scalar2=None,
                                op0=TT.mult)
                        else:
                            nc.vector.tensor_scalar(
                                out=og[:, gs, t, :], in0=hsb[:, t, :],
                                scalar1=rs[:, t : t + 1],
                                scalar2=cc[:, t : t + 1],
                                op0=TT.mult, op1=TT.add)
                    else:
                        nc.scalar.activation(
                            og[:, gs, t, :], hsb[:, t, :],
                            mybir.ActivationFunctionType.Identity,
                            bias=0.0 if CENTER else cc[:, t : t + 1],
                            scale=rs[:, t : t + 1])
                    if apply_affine:
                        nc.vector.tensor_mul(og[:, gs, t, :],
                                             og[:, gs, t, :], sb_gmb[:])
                        nc.vector.tensor_add(og[:, gs, t, :],
                                             og[:, gs, t, :], sb_btb[:])
                if g % 2 == 1:
                    # one store covers groups g-1 and g:
                    # DRAM row (g-1)*A + u*A + t*128 + p  <-  og[p, u, t, :]
                    out_view = out[a0 - A : a0 + A, :].rearrange(
                        "(u t p) d -> p u t d", p=128, u=2)
                    nc.sync.dma_start(out=out_view, in_=og[:])
                    del live[g - 1]
                    del live[g]

            for it in range(ngroups + 6):
                if it < ngroups:
                    stage_load(it)
                if 0 <= it - 3 < ngroups:
                    stage_copy(it - 3)
                if 0 <= it - 4 < ngroups:
                    stage_stat(it - 4)
                if 0 <= it - 5 < ngroups:
                    stage_coef(it - 5)
                if 0 <= it - 6 < ngroups:
                    stage_norm(it - 6)
                if 0 <= it - 1 < ngroups:
                    stage_front(it - 1)
                if 0 <= it - 2 < ngroups:
                    stage_back(it - 2)

    nc.compile()
    return nc


def _get_module(npc: int, apply_affine: bool, sim_safe_silu: bool = False):
    key = (npc, apply_affine, sim_safe_silu)
    if key not in _MODULE_CACHE:
        _MODULE_CACHE[key] = _build_module(npc, apply_affine, sim_safe_silu)
    return _MODULE_CACHE[key]


def _prep_inputs(z, x, emb, w1, b1, w2, b2, gamma, beta, npc, apply_affine):
    """Host-side folding/transposes; returns per-core in_maps."""
    import ml_dtypes

    st = ml_dtypes.bfloat16

    z = np.asarray(z)
    x = np.asarray(x, dtype=np.float32)
    n = z.shape[0]

    xT = np.zeros((128, n), dtype=np.float32)
    xT[0:3] = x.T
    xT[3] = 1.0
    xT = xT.astype(st)
    zi = np.asarray(z).astype(np.int64)
    ohT = (zi[None, :] == np.arange(NT, dtype=np.int64)[:, None]).astype(st)
    w1a = np.zeros((128, D), dtype=np.float32)
    w1a[0:3] = np.asarray(w1, np.float32)
    w1a[3] = np.asarray(b1, np.float32).reshape(D)
    w1c = w1a.astype(st)
    w2f = np.asarray(w2, np.float32)
    embf = (np.asarray(emb, np.float32)
            + np.asarray(b2, np.float32).reshape(1, D))
    if CENTER:
        # center each w2 row and emb row over the output dim d: then
        # mean_d(h) == 0 by construction (variance is translation-invariant,
        # so the LayerNorm shift term vanishes exactly)
        w2f = w2f - w2f.mean(axis=1, keepdims=True)
        embf = embf - embf.mean(axis=1, keepdims=True)
    w2a = np.stack([w2f[0:128], w2f[128:256]], axis=1).astype(st)
    emba = embf.astype(st)

    common = {"w1c": w1c, "w2a": w2a, "emba": emba}
    if apply_affine:
        common["gmb"] = np.broadcast_to(
            np.asarray(gamma, np.float32).reshape(1, D), (128, D)).copy()
        common["btb"] = np.broadcast_to(
            np.asarray(beta, np.float32).reshape(1, D), (128, D)).copy()

    in_maps = []
    for c in range(NCORES):
        s = slice(c * npc, (c + 1) * npc)
        m = {"xT": np.ascontiguousarray(xT[:, s]),
             "ohT": np.ascontiguousarray(ohT[:, s]), **common}
        in_maps.append(m)
    return in_maps


def _run(in_maps, nc, trace=False):
    from concourse.bass_interp import get_hw_module
    from concourse.bass_utils import run_bass_kernel_spmd

    old_m = nc.m
    nc.m = get_hw_module(nc.m)
    try:
        res = run_bass_kernel_spmd(
            nc, in_maps, core_ids=list(range(NCORES)), trace=trace
        )
    finally:
        nc.m = old_m
    return res


def kernel(z, x, emb, w1, b1, w2, b2, gamma, beta):
    z = np.asarray(z)
    x = np.asarray(x)
    assert z.shape[0] == N and x.shape == (N, 3), (z.shape, x.shape)

    apply_affine = not (
        np.all(np.asarray(gamma) == 1.0) and np.all(np.asarray(beta) == 0.0)
    )
    nc = _get_module(NPC, apply_affine)
    in_maps = _prep_inputs(z, x, emb, w1, b1, w2, b2, gamma, beta,
                           NPC, apply_affine)
    res = _run(in_maps, nc, trace=False)
    out = np.concatenate([np.asarray(r["out"]) for r in res.results], axis=0)
    return out.astype(np.float32)
